# revision 28
# baseline (speedup 1.0000x reference)
"""Trainium2 Bass kernel for a pre-norm transformer block (attention + MLP).

Sharding: batch (2) x query-block (4) across 8 cores. Each core computes
LN1 + K/V over its full batch (replicated within its 4-core group) and
attention / projection / MLP for its own 1024 query tokens. No collectives.

Device layouts (per core):
  xT  : LN1(x) feature-major bf16 [128c, 32mt, 4kt, 128t] (for K/V)
  xq8T/x2T8: fp8 kt-major [128c, 4kt, mt, 128t] via bf16 DMA-transpose
        stage + DVE convert (for the DoubleRow consumers Q and w1)
  kT  : per head-pair hp [128 (2 heads x 64d), 4096m] bf16
  v   : token-major [128m, 32mt, 8h, 65] fp8 (65th col = ones -> denom)
  S^T : [128m, 2h, 512n] PSUM (keys on partitions; head pair row-packed)
  P   : exp(S^T) fp8 -> A@V: out^T[65, n], lhsT=P^T [128,128] (FWL),
        moving=[V|1], accumulated over 32 m-tiles
  Q/w1/w2: fp8 DoubleRow (256-deep contraction per matmul); K/V/wp bf16

Schedule: the Act engine (exp on 33.5M scores/core) is critical.
Attention emits via generators (one softmax m-tile per yield):
  - phase A (LN1/QKV): attention(0,0) interleaved, LN pipelined one group
    ahead, loads on SP/Act queues (HWDGE), transposes on SP
  - K/Q for hp+1 interleaved into attention(hp) steps
  - order ends (3,0),(2,1),(3,1): MLP(nch0) overlaps att(2,1) (nch0 is
    complete after (3,0)); MLP(nch1) is a short tail after (3,1).
    Single flat pool scope (no mid-kernel pool-close barriers).
"""

import numpy as np
import ml_dtypes

B, N, C = 2, 4096, 512
H, D = 8, 64
HID = 2048
NQ = 1024
NCORES = 8
EPS = 1e-5
BF = ml_dtypes.bfloat16
F8 = ml_dtypes.float8_e4m3

_CACHE = {}


def _build_program(repeat=1):
    from concourse import bacc
    import concourse.bass as bass
    import concourse.mybir as mybir
    from concourse.tile import TileContext

    dt = mybir.dt
    AF = mybir.ActivationFunctionType
    ALU = mybir.AluOpType
    DR = mybir.MatmulPerfMode.DoubleRow

    nc = bacc.Bacc(None, target_bir_lowering=False)

    xfull = nc.dram_tensor("xfull", (N, C), dt.float32, kind="ExternalInput")
    xq = nc.dram_tensor("xq", (NQ, C), dt.float32, kind="ExternalInput")
    wq_d = nc.dram_tensor("wq_d", (128, 4, C), dt.float8e4, kind="ExternalInput")
    wk_d = nc.dram_tensor("wk_d", (128, 4, C), dt.bfloat16, kind="ExternalInput")
    wv_d = nc.dram_tensor("wv_d", (128, 4, C), dt.bfloat16, kind="ExternalInput")
    wp_d = nc.dram_tensor("wp_d", (128, 4, C), dt.bfloat16, kind="ExternalInput")
    w1_d = nc.dram_tensor("w1_d", (128, 4, HID), dt.float8e4, kind="ExternalInput")
    w2_d = nc.dram_tensor("w2_d", (128, 16, C), dt.float8e4, kind="ExternalInput")
    bq_d = nc.dram_tensor("bq_d", (128, 4), dt.float32, kind="ExternalInput")
    bk_d = nc.dram_tensor("bk_d", (128, 4), dt.float32, kind="ExternalInput")
    bv_d = nc.dram_tensor("bv_d", (C,), dt.float32, kind="ExternalInput")
    bp_d = nc.dram_tensor("bp_d", (C,), dt.float32, kind="ExternalInput")
    b1_d = nc.dram_tensor("b1_d", (128, 16), dt.float32, kind="ExternalInput")
    b2_d = nc.dram_tensor("b2_d", (C,), dt.float32, kind="ExternalInput")
    y = nc.dram_tensor("y", (NQ, C), dt.float32, kind="ExternalOutput")

    xq_t = xq.rearrange("(i p) c -> p i c", p=128)
    y_t = y.rearrange("(i p) c -> p i c", p=128)

    import contextlib
    with TileContext(nc) as tc:
      with (tc.For_i(0, repeat, 1) if repeat > 1 else contextlib.nullcontext()):
        R = "r0_"
        with tc.tile_pool(name=R + "pers", bufs=1) as pers, \
             tc.tile_pool(name=R + "stat", bufs=4) as statp, \
             tc.tile_pool(name=R + "stream", bufs=3) as stream, \
             tc.tile_pool(name=R + "kq", bufs=2) as kqp, \
             tc.tile_pool(name=R + "ptp", bufs=3) as ptp, \
             tc.tile_pool(name=R + "pall", bufs=1, space="PSUM") as pall:

            eps_t = pers.tile([128, 1], dt.float32, name=R + "eps")
            nc.vector.memset(eps_t, EPS)
            xq_sb = pers.tile([128, 8, C], dt.float32, name=R + "xq_sb")
            nc.sync.dma_start(out=xq_sb, in_=xq_t[:])
            bq_sb = pers.tile([128, 4], dt.float32, name=R + "bq_sb")
            bk_sb = pers.tile([128, 4], dt.float32, name=R + "bk_sb")
            bv_sb = pers.tile([128, 8, 64], dt.bfloat16, name=R + "bv_sb")
            bp_sb = pers.tile([128, C], dt.float32, name=R + "bp_sb")
            b1_sb = pers.tile([128, 16], dt.float32, name=R + "b1_sb")
            b2_sb = pers.tile([128, C], dt.float32, name=R + "b2_sb")
            nc.sync.dma_start(out=bq_sb, in_=bq_d[:])
            nc.sync.dma_start(out=bk_sb, in_=bk_d[:])
            nc.sync.dma_start(out=b1_sb, in_=b1_d[:])
            nc.gpsimd.dma_start(out=bv_sb, in_=bass.AP(tensor=bv_d, offset=0, ap=[[0, 128], [1, C]]))
            nc.gpsimd.dma_start(out=bp_sb, in_=bass.AP(tensor=bp_d, offset=0, ap=[[0, 128], [1, C]]))
            nc.gpsimd.dma_start(out=b2_sb, in_=bass.AP(tensor=b2_d, offset=0, ap=[[0, 128], [1, C]]))

            # weights (fp8 for DR consumers: q/w1/w2; bf16 for k/v/wp)
            wq_sb = pers.tile([128, 4, C], dt.float8e4, name=R + "wq_sb")
            wk_sb = pers.tile([128, 4, C], dt.bfloat16, name=R + "wk_sb")
            wv_sb = pers.tile([128, 4, C], dt.bfloat16, name=R + "wv_sb")
            wp_sb = pers.tile([128, 4, C], dt.bfloat16, name=R + "wp_sb")
            w1_sb = pers.tile([128, 4, HID], dt.float8e4, name=R + "w1_sb")
            w2_sb = pers.tile([128, 16, C], dt.float8e4, name=R + "w2_sb")
            nc.gpsimd.dma_start(out=wq_sb, in_=wq_d[:])
            nc.gpsimd.dma_start(out=wk_sb, in_=wk_d[:])
            nc.gpsimd.dma_start(out=wv_sb, in_=wv_d[:])
            nc.gpsimd.dma_start(out=wp_sb, in_=wp_d[:])
            nc.gpsimd.dma_start(out=w1_sb, in_=w1_d[:])
            nc.gpsimd.dma_start(out=w2_sb, in_=w2_d[:])

            # transposed activations: xT bf16 (baseline mt,kt layout) for K/V;
            # fp8 kt-major for the DoubleRow consumers (Q, w1)
            xT = pers.tile([128, 32, 4, 128], dt.bfloat16, name=R + "xT")
            xq8T = pers.tile([128, 4, 8, 128], dt.float8e4, name=R + "xq8T")
            x2T8 = pers.tile([128, 4, 8, 128], dt.float8e4, name=R + "x2T8")

            # attention persistents
            v_sb = pers.tile([128, 32, H, 65], dt.float8e4, name=R + "v_sb")
            nc.vector.memset(v_sb[:, :, :, 64:65], 1.0)
            ao_nm = pers.tile([128, 8, H, 64], dt.bfloat16, name=R + "ao_nm")
            aoT = pers.tile([128, 8, 4, 128], dt.bfloat16, name=R + "aoT")

            def ln_group(src_dram_or_sb, g, dst, gdst, tag, from_sbuf=False,
                         fp8=False):
                """LN over 4 token-tiles -> transpose. fp8=False: direct bf16
                transpose into dst [128c, mt, 4kt, 128t] (mt slice 4*gdst..).
                fp8=True: bf16 stage -> DVE fp8 convert into dst
                [128c, 4kt, mt, 128t]."""
                if from_sbuf:
                    xt4 = src_dram_or_sb
                else:
                    xt4 = stream.tile([128, 4, C], dt.float32, tag="lnx", bufs=2, name=f"{R}{tag}x{g}")
                    ldeng = nc.sync if g % 2 == 0 else nc.scalar
                    ldeng.dma_start(out=xt4, in_=src_dram_or_sb)
                xn4 = stream.tile([128, 4, C], dt.bfloat16, tag="lnn", bufs=3, name=f"{R}{tag}n{g}")
                mv4 = statp.tile([128, 4, 2], dt.float32, tag="lnmv", name=f"{R}{tag}mv{g}")
                for j in range(4):
                    stats = statp.tile([128, 6], dt.float32, tag="lnst", name=f"{R}{tag}st{4*g+j}")
                    nc.vector.bn_stats(stats, xt4[:, j, :])
                    nc.vector.bn_aggr(mv4[:, j, :], stats)
                # rstd = exp(-0.5*ln(var+eps)): keeps Act on the single ln/exp table
                lnv = statp.tile([128, 4], dt.float32, tag="lnlv", name=f"{R}{tag}lv{g}")
                nc.scalar.activation(lnv, mv4[:, :, 1], AF.Ln, bias=eps_t)
                rstd4 = statp.tile([128, 4], dt.float32, tag="lnrs", name=f"{R}{tag}rs{g}")
                nc.scalar.activation(rstd4, lnv, AF.Exp, scale=-0.5)
                nmr4 = statp.tile([128, 4], dt.float32, tag="lnnm", name=f"{R}{tag}nm{g}")
                nc.vector.tensor_tensor(out=nmr4, in0=mv4[:, :, 0], in1=rstd4, op=ALU.mult)
                nc.vector.tensor_scalar(nmr4, nmr4, -1.0, None, ALU.mult)
                for j in range(4):
                    nc.vector.tensor_scalar(xn4[:, j, :], xt4[:, j, :],
                                            rstd4[:, j:j + 1], nmr4[:, j:j + 1],
                                            ALU.mult, ALU.add)
                if not fp8:
                    nc.sync.dma_start(out=dst[:, 4 * gdst:4 * gdst + 4, :, :],
                                      in_=xn4, transpose=True)
                    return
                # transpose writes (mt, kt, t) free-dim order; fp8 dst wants (kt, mt, t)
                stg = stream.tile([128, 4, 4, 128], dt.bfloat16, tag="stg", bufs=2,
                                  name=f"{R}{tag}s{g}")
                nc.sync.dma_start(out=stg, in_=xn4, transpose=True)
                nc.vector.tensor_scalar(dst[:, :, 4 * gdst:4 * gdst + 4, :],
                                        stg.rearrange("p m k t -> p k m t"),
                                        0.0, None, ALU.add)

            # ---- attention generator: one yield per m-tile step (34 total) ----
            kqt = {}

            def gen_attention(hp, nch):
                kT, qT = kqt[hp]
                nsl = slice(nch * 512, (nch + 1) * 512)
                po = [pall.tile([128, 4, 65], dt.float32, tag=f"po{h}",
                                name=f"{R}po{hp}_{nch}_{h}") for h in range(2)]
                pts = {}
                for mt in range(33):
                    if mt < 32:
                        msl = slice(mt * 128, (mt + 1) * 128)
                        ps_s = pall.tile([128, 2, 512], dt.float32, bufs=2,
                                         tag="ps_s", name=f"{R}ps_s{hp}_{nch}_{mt}")
                        nc.tensor.matmul(ps_s[:, 0, :], kT[0:64, msl], qT[0:64, nsl],
                                         start=True, stop=True)
                        nc.tensor.matmul(ps_s[:, 1, :], kT[64:128, msl], qT[64:128, nsl],
                                         start=True, stop=True, tile_position=(64, 0))
                        pt = ptp.tile([128, 2, 512], dt.float8e4, bufs=4, tag="pt",
                                      name=f"{R}pt{hp}_{nch}_{mt}")
                        nc.scalar.activation(pt, ps_s, AF.Exp, scale=float(D) ** -0.5)
                        pts[mt] = pt
                    if mt >= 1:
                        ptm = pts.pop(mt - 1)
                        for h in range(2):
                            for c4 in range(4):
                                nc.tensor.matmul(
                                    po[h][:, c4, :],
                                    ptm[:, h, c4 * 128:(c4 + 1) * 128],
                                    v_sb[:, mt - 1, 2 * hp + h, :],
                                    start=(mt - 1 == 0 and c4 == 0),
                                    stop=(mt - 1 == 31 and c4 == 3),
                                    skip_group_check=True)
                    yield
                for h in range(2):
                    for c4 in range(4):
                        nb = nch * 4 + c4
                        rden = statp.tile([128, 1], dt.float32, bufs=4, tag="rden",
                                          name=f"{R}rden{hp}_{nch}_{h}_{c4}")
                        nc.vector.reciprocal(rden, po[h][:, c4, 64:65])
                        nc.vector.tensor_scalar(ao_nm[:, nb, 2 * hp + h, :],
                                                po[h][:, c4, 0:64], rden,
                                                None, ALU.mult)
                yield

            def drive(gen, n=None):
                if n is None:
                    for _ in gen:
                        pass
                else:
                    for _ in range(n):
                        next(gen, None)

            def drive_with(gen, units, every=1):
                i = 0
                for _ in gen:
                    if units and i % every == 0:
                        units.pop(0)()
                    i += 1
                while units:
                    units.pop(0)()

            def emit_v(mt):
                ps_v = pall.tile([128, C], dt.float32, bufs=2, tag="psmall",
                                 name=f"{R}ps_v{mt}")
                for kt in range(4):
                    nc.tensor.matmul(ps_v, xT[:, mt, kt, :],
                                     wv_sb[:, kt, :], start=(kt == 0), stop=(kt == 3))
                nc.vector.tensor_tensor(
                    out=v_sb[:, mt, :, 0:64],
                    in0=ps_v.rearrange("p (h d) -> p h d", h=H),
                    in1=bv_sb, op=ALU.add)

            def emit_k_chunk(hp, kT, nch):
                ps_k = pall.tile([128, 512], dt.float32, bufs=2, tag="psmall",
                                 name=f"{R}ps_k{hp}_{nch}")
                for kt in range(4):
                    nc.tensor.matmul(ps_k, wk_sb[:, kt, hp * 128:(hp + 1) * 128],
                                     xT[:, 4 * nch:4 * nch + 4, kt, :],
                                     start=(kt == 0), stop=(kt == 3))
                nc.vector.tensor_scalar(kT[:, nch * 512:(nch + 1) * 512],
                                        ps_k, bk_sb[:, hp:hp + 1], None, ALU.add)

            def emit_q_chunk(hp, qT, nch):
                ps_q = pall.tile([128, 512], dt.float32, bufs=2, tag="psmall",
                                 name=f"{R}ps_q{hp}_{nch}")
                for kp in range(2):
                    nc.tensor.matmul(ps_q, wq_sb[:, 2 * kp:2 * kp + 2, hp * 128:(hp + 1) * 128],
                                     xq8T[:, 2 * kp:2 * kp + 2, 4 * nch:4 * nch + 4, :],
                                     start=(kp == 0), stop=(kp == 1), perf_mode=DR)
                nc.vector.tensor_scalar(qT[:, nch * 512:(nch + 1) * 512],
                                        ps_q, bq_sb[:, hp:hp + 1], None, ALU.add)

            def kq_units(hp):
                kT = kqp.tile([128, N], dt.bfloat16, tag="kT", name=f"{R}kT{hp}")
                qT = kqp.tile([128, NQ], dt.bfloat16, tag="qT", name=f"{R}qT{hp}")
                kqt[hp] = (kT, qT)
                units = [lambda n=n: emit_k_chunk(hp, kT, n) for n in range(8)]
                units += [lambda n=n: emit_q_chunk(hp, qT, n) for n in range(2)]
                return units

            # ======== phase A: LN1 + QKV, attention(0,0) interleaved ========
            kT0 = kqp.tile([128, N], dt.bfloat16, tag="kT", name=f"{R}kT0")
            qT0 = kqp.tile([128, NQ], dt.bfloat16, tag="qT", name=f"{R}qT0")
            kqt[0] = (kT0, qT0)

            xf4_t = xfull.rearrange("(gr j p) c -> p gr j c", p=128, j=4)
            xq4_t = xq.rearrange("(gr j p) c -> p gr j c", p=128, j=4)

            ln_group(xf4_t[:, 0, :, :], 0, xT, 0, "l1")
            for g in range(2):
                ln_group(xq4_t[:, g, :, :], g + 1, xq8T, g, "lq", fp8=True)
            emit_q_chunk(0, qT0, 0)
            emit_q_chunk(0, qT0, 1)

            g00 = gen_attention(0, 0)
            for g in range(8):
                if g < 7:
                    ln_group(xf4_t[:, g + 1, :, :], g + 1, xT, g + 1, "l1")
                emit_k_chunk(0, kT0, g)
                for mt in range(4 * g, 4 * g + 4):
                    drive(g00, 1)
                    emit_v(mt)
            drive(g00)

            drive_with(gen_attention(0, 1), kq_units(1), every=3)
            drive_with(gen_attention(1, 0), kq_units(2), every=3)
            drive_with(gen_attention(1, 1), kq_units(3), every=3)
            drive(gen_attention(2, 0))

            # ======== attention tail + wp/LN2/MLP interleaved ========
            hsb = {}

            def t_unit(nb):
                nc.sync.dma_start(out=aoT[:, nb, :, :], in_=ao_nm[:, nb, :, :],
                                  transpose=True)

            def wp_unit(ns):
                ps_p = pall.tile([128, C], dt.float32, bufs=2, tag="psmall",
                                 name=f"{R}ps_p{ns}")
                for blk in range(4):
                    nc.tensor.matmul(ps_p, aoT[:, ns, blk, :], wp_sb[:, blk, :],
                                     start=(blk == 0), stop=(blk == 3))
                nc.vector.tensor_tensor(out=xq_sb[:, ns, :], in0=xq_sb[:, ns, :],
                                        in1=ps_p, op=ALU.add)
                nc.vector.tensor_tensor(out=xq_sb[:, ns, :], in0=xq_sb[:, ns, :],
                                        in1=bp_sb, op=ALU.add)

            def ln2_unit(g):
                ln_group(xq_sb.rearrange("p (gr j) c -> p gr j c", j=4)[:, g, :, :],
                         g, x2T8, g, "l2", from_sbuf=True, fp8=True)

            def h_unit(nch, pt_i):
                if nch not in hsb:
                    hsb[nch] = stream.tile([128, 16, 512], dt.float8e4, tag="h_sb",
                                           bufs=2, name=f"{R}h_sb{nch}")
                h_sb = hsb[nch]
                ps_h = pall.tile([128, 512], dt.float32, bufs=2, tag="psmall",
                                 name=f"{R}ps_h{pt_i}_{nch}")
                for kp in range(2):
                    nc.tensor.matmul(ps_h, w1_sb[:, 2 * kp:2 * kp + 2, pt_i * 128:(pt_i + 1) * 128],
                                     x2T8[:, 2 * kp:2 * kp + 2, 4 * nch:4 * nch + 4, :],
                                     start=(kp == 0), stop=(kp == 1), perf_mode=DR)
                nc.vector.tensor_scalar(h_sb[:, pt_i, :],
                                        ps_h, b1_sb[:, pt_i:pt_i + 1], 0.0,
                                        ALU.add, ALU.max)

            def m_unit(nch, ns, half):
                h_sb = hsb[nch]
                qsl = slice((ns - 4 * nch) * 128, (ns - 4 * nch + 1) * 128)
                if half == 0:
                    ps_m = pall.tile([128, C], dt.float32, bufs=2, tag="psmall",
                                     name=f"{R}ps_m{ns}")
                    hsb[("pm", ns)] = ps_m
                    for kp in range(4):
                        nc.tensor.matmul(ps_m, h_sb[:, 2 * kp:2 * kp + 2, qsl],
                                         w2_sb[:, 2 * kp:2 * kp + 2, :],
                                         start=(kp == 0), stop=False, perf_mode=DR)
                else:
                    ps_m = hsb.pop(("pm", ns))
                    for kp in range(4, 8):
                        nc.tensor.matmul(ps_m, h_sb[:, 2 * kp:2 * kp + 2, qsl],
                                         w2_sb[:, 2 * kp:2 * kp + 2, :],
                                         start=False, stop=(kp == 7), perf_mode=DR)
                    ot = stream.tile([128, C], dt.float32, tag="out", name=f"{R}out{ns}")
                    nc.vector.tensor_tensor(out=ot, in0=ps_m, in1=xq_sb[:, ns, :], op=ALU.add)
                    nc.vector.tensor_tensor(out=ot, in0=ot, in1=b2_sb, op=ALU.add)
                    nc.sync.dma_start(out=y_t[:, ns, :], in_=ot)

            def mlp_units(nch):
                units = [lambda nb=nb: t_unit(nb) for nb in range(4 * nch, 4 * nch + 4)]
                units += [lambda ns=ns: wp_unit(ns) for ns in range(4 * nch, 4 * nch + 4)]
                units += [lambda: ln2_unit(nch)]
                units += [lambda p=p: h_unit(nch, p) for p in range(16)]
                for ns in range(4 * nch, 4 * nch + 4):
                    units += [lambda ns=ns: m_unit(nch, ns, 0),
                              lambda ns=ns: m_unit(nch, ns, 1)]
                return units

            # nch0 attention completes at (3,0) -> MLP(nch0) overlaps (2,1);
            # MLP(nch1) needs (3,1) itself -> short tail after it.
            drive(gen_attention(3, 0))
            drive_with(gen_attention(2, 1), mlp_units(0))
            drive(gen_attention(3, 1))
            for u in mlp_units(1):
                u()

    # Restrict the act-table placement pass to the single table holding both
    # Ln and Exp (others emptied, not removed, to keep act_func_set_id stable).
    import concourse.bacc as bacc_mod
    orig_tables = bacc_mod.get_activation_tables
    keep = "natural_log_exp_and_others"

    def _only_nl_exp(arch):
        return {k: (v if k == keep else set())
                for k, v in orig_tables(arch).items()}

    bacc_mod.get_activation_tables = _only_nl_exp
    try:
        nc.finalize()
    finally:
        bacc_mod.get_activation_tables = orig_tables
    return nc


def _prepare_host(inputs):
    f32 = np.float32
    x = np.asarray(inputs["x"], f32)
    ln1_w = np.asarray(inputs["ln1_w"], f32); ln1_b = np.asarray(inputs["ln1_b"], f32)
    ln2_w = np.asarray(inputs["ln2_w"], f32); ln2_b = np.asarray(inputs["ln2_b"], f32)
    wq = np.asarray(inputs["wq"], f32); wkv = np.asarray(inputs["wkv"], f32)
    wp = np.asarray(inputs["wp"], f32); bp = np.asarray(inputs["bp"], f32)
    w1 = np.asarray(inputs["w1"], f32); b1 = np.asarray(inputs["b1"], f32)
    w2 = np.asarray(inputs["w2"], f32); b2 = np.asarray(inputs["b2"], f32)

    wq_f = ln1_w[:, None] * wq
    wkv_f = ln1_w[:, None] * wkv
    w1_f = ln2_w[:, None] * w1
    bq_f = ln1_b @ wq
    bkv_f = ln1_b @ wkv
    b1_f = b1 + ln2_b @ w1

    def kmaj(w, cols, kt, dtype):
        return np.ascontiguousarray(w.reshape(kt, 128, cols).transpose(1, 0, 2)).astype(dtype)

    shared = dict(
        wq_d=kmaj(wq_f, C, 4, F8),
        wk_d=kmaj(wkv_f[:, :C], C, 4, BF),
        wv_d=kmaj(wkv_f[:, C:], C, 4, BF),
        wp_d=kmaj(wp, C, 4, BF),
        w1_d=kmaj(w1_f, HID, 4, F8),
        w2_d=np.ascontiguousarray(w2.reshape(16, 128, C).transpose(1, 0, 2)).astype(F8),
        bq_d=np.ascontiguousarray(bq_f.reshape(4, 128).T).astype(f32),
        bk_d=np.ascontiguousarray(bkv_f[:C].reshape(4, 128).T).astype(f32),
        bv_d=np.ascontiguousarray(bkv_f[C:]).astype(f32),
        bp_d=np.ascontiguousarray(bp).astype(f32),
        b1_d=np.ascontiguousarray(b1_f.reshape(16, 128).T).astype(f32),
        b2_d=np.ascontiguousarray(b2).astype(f32),
    )

    in_maps = []
    for core in range(NCORES):
        bi, qi = divmod(core, 4)
        in_maps.append(dict(shared,
                            xfull=np.ascontiguousarray(x[bi]),
                            xq=np.ascontiguousarray(x[bi, qi * NQ:(qi + 1) * NQ])))
    return in_maps


def _make_runner(nc):
    """Persistent jitted SPMD executor for `nc` (mirrors bass2jax.run_bass_via_pjrt
    but keeps the jitted function + avoids per-call retrace)."""
    import jax
    import numpy as jnp_np
    from jax.sharding import Mesh, PartitionSpec
    from jax.experimental.shard_map import shard_map
    import concourse.mybir as mybir
    from concourse import bass2jax

    bass2jax.install_neuronx_cc_hook()

    partition_name = nc.partition_id_tensor.name if nc.partition_id_tensor else None
    in_names, out_names, out_avals = [], [], []
    for alloc in nc.m.functions[0].allocations:
        if not isinstance(alloc, mybir.MemoryLocationSet):
            continue
        name = alloc.memorylocations[0].name
        if alloc.kind == "ExternalInput":
            if name != partition_name:
                in_names.append(name)
        elif alloc.kind == "ExternalOutput":
            out_names.append(name)
            out_avals.append(jax.core.ShapedArray(tuple(alloc.tensor_shape),
                                                  mybir.dt.np(alloc.dtype)))
    n_params = len(in_names)
    all_names = in_names + out_names
    if partition_name is not None:
        all_names = all_names + [partition_name]

    def _body(*args):
        operands = list(args)
        if partition_name is not None:
            operands.append(bass2jax.partition_id_tensor())
        outs = bass2jax._bass_exec_p.bind(
            *operands,
            out_avals=tuple(out_avals),
            in_names=tuple(all_names),
            out_names=tuple(out_names),
            lowering_input_output_aliases=(),
            sim_require_finite=True,
            sim_require_nnan=True,
            nc=nc,
        )
        return tuple(outs)

    devices = jax.devices()[:NCORES]
    mesh = Mesh(np.asarray(devices), ("core",))
    n_outs = len(out_names)
    sharded = jax.jit(
        shard_map(_body, mesh=mesh,
                  in_specs=(PartitionSpec("core"),) * (n_params + n_outs),
                  out_specs=(PartitionSpec("core"),) * n_outs,
                  check_rep=False),
        keep_unused=True,
    )

    def run(in_maps):
        concat_in = [np.concatenate([np.asarray(in_maps[c][name]) for c in range(NCORES)], axis=0)
                     for name in in_names]
        zeros = [np.zeros((NCORES * a.shape[0], *a.shape[1:]), a.dtype) for a in out_avals]
        out_arrs = sharded(*concat_in, *zeros)
        return [{name: np.asarray(out_arrs[i]).reshape(NCORES, *out_avals[i].shape)[c]
                 for i, name in enumerate(out_names)}
                for c in range(NCORES)]

    run.sharded = sharded
    run.in_names = in_names
    run.out_names = out_names
    run.out_avals = out_avals
    return run


def get_runner(repeat=1):
    key = f"runner{repeat}"
    if key not in _CACHE:
        _CACHE[key] = _make_runner(_build_program(repeat=repeat))
    return _CACHE[key]


def kernel(**inputs):
    runner = get_runner()
    in_maps = _prepare_host(inputs)
    results = runner(in_maps)
    out = np.empty((B, N, C), np.float32)
    for core in range(NCORES):
        bi, qi = divmod(core, 4)
        out[bi, qi * NQ:(qi + 1) * NQ] = results[core]["y"]
    return out
